# revision 15
# baseline (speedup 1.0000x reference)
"""Trainium2 Bass kernel for nn_MetaSelectTarget (FPN level assignment).

Strategy (v3):
  - Data-parallel over batch: B=8 images -> 8 NeuronCores, one image each.
  - 128 partitions: box g lives on partitions g and g+64, each half handling
    half of the window rows.  Unified column layout packs all levels' window
    cells into 70 slots.
  - Host-side layout prep (pure data movement): cls and regr are concatenated
    into one [NLOC, 84] array so the 12 row-block gathers fetch class AND
    regr data together; cls is also transposed to [80, NLOC] so the level-0
    per-gt-class probabilities come from ONE span gather (start =
    label*NLOC + y1*fw + x1, static in-span offsets b*fw+j).
  - psel for levels 1-4 via one-hot multiply + class reduce.
  - Everything after the gathers is split per level so compute pipelines
    behind the serial SWDGE descriptor generation; only level-4's small
    tail trails the last gather.
  - Tail: halves combined with a DVE stream_shuffle (partition crossbar).
"""

import numpy as np

import concourse.bass as bass
import concourse.bacc as bacc
import concourse.tile as tile
from concourse import mybir
from contextlib import ExitStack

f32 = mybir.dt.float32
i32 = mybir.dt.int32
u16 = mybir.dt.uint16
AF = mybir.ActivationFunctionType
OP = mybir.AluOpType
AX = mybir.AxisListType

G = 64
C = 80
CC = 84                   # combined row: 80 cls + 4 regr
FS = [(128, 128), (64, 64), (32, 32), (16, 16), (8, 8)]
STRIDES = [8.0, 16.0, 32.0, 64.0, 128.0]
ROWOFS = [0, 16384, 20480, 21504, 21760]
NLOC = 21824
W = [9, 5, 3, 2, 2]
NH = [5, 3, 2, 1, 1]
EPS = 1e-7
BIG = 1e7
BIG2 = 16.0
NLEV = 5

_cols = []      # (level, block, slot)
for _l in range(NLEV):
    for _b in range(NH[_l]):
        for _j in range(W[_l]):
            _cols.append((_l, _b, _j))
NW = len(_cols)           # 70
assert NW == 70
_lvl_cols = [sum(NH[l] * W[l] for l in range(lv)) for lv in range(NLEV + 1)]
_J0 = [(l, b, k) for k, (l, b, j) in enumerate(_cols) if j == 0]
assert len(_J0) == 12


def _wrap_idx(mapv):
    n = len(mapv)
    s = (n + 15) // 16
    a = np.zeros((128, s), np.uint16)
    for i, m in enumerate(mapv):
        a[np.arange(8) * 16 + i % 16, i // 16] = m
    return a


def build_nc(num_devices=8):
    nc = bacc.Bacc("TRN2", target_bir_lowering=False, num_devices=num_devices)

    comb_b = nc.dram_tensor("comb_b", [NLOC, CC], f32, kind="ExternalInput")
    clsT_b = nc.dram_tensor("clsT_b", [C, NLOC], f32, kind="ExternalInput")
    gt_b = nc.dram_tensor("gt_b", [G, 5], f32, kind="ExternalInput")
    idx_b = nc.dram_tensor("idx_b", [128, 14], i32, kind="ExternalInput")
    out_lvl = nc.dram_tensor("out_lvl", [G], i32, kind="ExternalOutput")

    comb_flat = comb_b.ap().rearrange("n c -> (n c)")[None, :]
    clsT_flat = clsT_b.ap().rearrange("n c -> (n c)")[None, :]

    # ---- inline constants --------------------------------------------------
    recip = np.zeros((128, NLEV, 4), np.float32)
    maskF = np.zeros((128, NLEV, 4), np.float32)
    maskC = np.zeros((128, NLEV, 4), np.float32)
    clo = np.zeros((128, NLEV, 4), np.float32)
    chi = np.zeros((128, NLEV, 4), np.float32)
    shi = np.zeros((128, NLEV, 4), np.float32)
    for l in range(NLEV):
        fh, fw = FS[l]
        w = W[l]
        recip[:, l, :] = 1.0 / STRIDES[l]
        maskF[:, l, 0] = maskF[:, l, 1] = 1.0
        maskC[:, l, 2] = maskC[:, l, 3] = 1.0
        clo[:, l, :] = [0.0, 0.0, 1.0, 1.0]
        chi[:, l, :] = [fw - 1, fh - 1, fw, fh]
        shi[:, l, :] = [fw - w, fh - w, 1e9, 1e9]

    r74 = np.zeros((128, NW), np.float32)
    j74 = np.zeros((128, NW), np.float32)
    inv4s296 = np.zeros((128, 4 * NW), np.float32)
    for k, (l, b, j) in enumerate(_cols):
        r74[0:64, k] = b
        r74[64:128, k] = NH[l] + b
        j74[:, k] = j
        for ch in range(4):
            inv4s296[:, ch * NW + k] = 1.0 / (4.0 * STRIDES[l])

    r12 = np.zeros((128, 12), np.float32)
    fw12 = np.zeros((128, 12), np.float32)
    fhm112 = np.zeros((128, 12), np.float32)
    rofs12 = np.zeros((128, 12), np.float32)
    for s, (l, b, k) in enumerate(_J0):
        r12[0:64, s] = b
        r12[64:128, s] = NH[l] + b
        fw12[:, s] = FS[l][1]
        fhm112[:, s] = FS[l][0] - 1
        rofs12[:, s] = ROWOFS[l]

    cconst = np.tile(np.arange(C, dtype=np.float32), (128, 1))
    clb5 = np.tile(np.arange(NLEV, dtype=np.float32) + BIG2, (128, 1))
    constm1 = np.full((128, 1), -1.0, np.float32)
    consteps = np.full((128, 1), EPS, np.float32)
    half640 = np.zeros((128, 1), np.float32)
    half640[64:128, 0] = float(NH[0] * FS[0][1])

    consts = np.concatenate(
        [recip.reshape(128, -1), maskF.reshape(128, -1), maskC.reshape(128, -1),
         clo.reshape(128, -1), chi.reshape(128, -1), shi.reshape(128, -1),
         r74, j74, inv4s296,
         r12, fw12, fhm112, rofs12,
         cconst, clb5, constm1, consteps, half640], axis=1)
    t_consts = nc.inline_tensor(consts, "c_all")
    NCONST = consts.shape[1]

    xsys_map = []
    for q in range(6):
        for k, (l, b, j) in enumerate(_cols):
            xsys_map.append([4 * l + 0, 4 * l + 1, 20 + 4 * l + 0, 20 + 4 * l + 1,
                             20 + 4 * l + 2, 20 + 4 * l + 3][q])
    for s, (l, b, k) in enumerate(_J0):
        xsys_map.append(4 * l + 0)
    for s, (l, b, k) in enumerate(_J0):
        xsys_map.append(4 * l + 1)
    t_xsys_idx_s = nc.inline_tensor(_wrap_idx(xsys_map[6 * NW:]), "xsys_idx_s")
    t_xsys_idx_b = nc.inline_tensor(_wrap_idx(xsys_map[:6 * NW]), "xsys_idx_b")
    NXS = len(xsys_map)  # 444

    NPW = NW * CC
    NF0 = NW * C

    with tile.TileContext(nc) as tc, ExitStack() as ctx:
        pc = ctx.enter_context(tc.tile_pool(name="pc", bufs=1))

        # host-computed gather descriptors first: they gate the gathers
        IDXT = pc.tile([128, 14], i32)
        nc.sync.dma_start(IDXT[:], idx_b[:])
        GT = pc.tile([128, 5], f32)
        nc.sync.dma_start(GT[0:64, :], gt_b[:])
        nc.gpsimd.dma_start(GT[64:128, :], gt_b[:])
        CST = pc.tile([128, NCONST], f32)
        nc.sync.dma_start(CST[:, 0:120], t_consts[:, 0:120])
        nc.sync.dma_start(CST[:, 120:NCONST], t_consts[:, 120:NCONST])
        XSYS_IDX_S = pc.tile([128, 2], u16)
        XSYS_IDX_B = pc.tile([128, (6 * NW + 15) // 16], u16)
        nc.sync.dma_start(XSYS_IDX_S[:], t_xsys_idx_s[:])
        nc.sync.dma_start(XSYS_IDX_B[:], t_xsys_idx_b[:])
        WARM = pc.tile([1, 2], f32)
        nc.vector.memset(WARM[:], 0.5)
        nc.scalar.activation(WARM[:, 0:1], WARM[:, 0:1], AF.Ln)
        nc.scalar.activation(WARM[:, 1:2], WARM[:, 1:2], AF.Square)

        off = 0
        def _cview(n):
            nonlocal off
            v = CST[:, off:off + n]
            off += n
            return v
        RECIP = _cview(NLEV * 4)
        MASKF = _cview(NLEV * 4)
        MASKC = _cview(NLEV * 4)
        CLO = _cview(NLEV * 4)
        CHI = _cview(NLEV * 4)
        SHI = _cview(NLEV * 4)
        R74 = _cview(NW)
        J74 = _cview(NW)
        INV4S296 = _cview(4 * NW)
        R12 = _cview(12)
        FW12 = _cview(12)
        FHM112 = _cview(12)
        ROFS12 = _cview(12)
        CCONST = _cview(C)
        CLB5 = _cview(NLEV)
        CONSTM1 = _cview(1)
        CONSTEPS = _cview(1)
        HALF640 = _cview(1)

        # ---- box math: critical path to the gathers -------------------------
        GTSW = pc.tile([128, 4], f32)
        nc.vector.tensor_copy(GTSW[:, 0:2], GT[:, 2:4])
        nc.vector.tensor_copy(GTSW[:, 2:4], GT[:, 0:2])
        Q = pc.tile([128, 4], f32)
        nc.vector.tensor_scalar(Q[:], GTSW[:], 0.4, None, OP.mult)
        nc.vector.scalar_tensor_tensor(Q[:], GT[:, 0:4], 0.6, Q[:], OP.mult, OP.add)

        SVVR = pc.tile([128, 40], f32)
        SV = SVVR[:, 0:20]
        VR = SVVR[:, 20:40]
        V = pc.tile([128, NLEV * 4], f32)
        nc.vector.tensor_tensor(
            out=V[:].rearrange("g (l j) -> g l j", j=4),
            in0=Q[:, None, :].to_broadcast([128, NLEV, 4]),
            in1=RECIP.rearrange("g (l j) -> g l j", j=4),
            op=OP.mult,
        )
        VI = pc.tile([128, NLEV * 4], i32)
        nc.vector.tensor_copy(VI[:], V[:])
        VF = pc.tile([128, NLEV * 4], f32)
        nc.vector.tensor_copy(VF[:], VI[:])
        GG = pc.tile([128, NLEV * 4], f32)
        nc.vector.tensor_tensor(out=GG[:], in0=VF[:], in1=V[:], op=OP.is_gt)
        LL = pc.tile([128, NLEV * 4], f32)
        nc.vector.tensor_tensor(out=LL[:], in0=VF[:], in1=V[:], op=OP.is_lt)
        nc.vector.tensor_tensor(out=GG[:], in0=GG[:], in1=MASKF, op=OP.mult)
        nc.vector.tensor_tensor(out=LL[:], in0=LL[:], in1=MASKC, op=OP.mult)
        nc.vector.tensor_tensor(out=VR, in0=VF[:], in1=GG[:], op=OP.subtract)
        nc.vector.tensor_tensor(out=VR, in0=VR, in1=LL[:], op=OP.add)
        nc.vector.tensor_tensor(out=VR, in0=VR, in1=CLO, op=OP.max)
        nc.vector.tensor_tensor(out=VR, in0=VR, in1=CHI, op=OP.min)
        nc.vector.tensor_tensor(out=SV, in0=VR, in1=SHI, op=OP.min)

        XSYS = pc.tile([128, NXS], f32)
        nc.gpsimd.indirect_copy(XSYS[:, 6 * NW:NXS], SVVR[:], XSYS_IDX_S[:], True)
        RIC = IDXT[:, 0:12]
        PSIDX = IDXT[:, 12:13]
        PSIDX1 = IDXT[:, 13:14]

        # ---- gathers: L0 blocks first, then psel span + big broadcast, then
        # levels 1-4 (order == compute order so everything pipelines) ---------
        PWALL = pc.tile([128, NPW], f32)
        PWv = PWALL[:].rearrange("g (k c) -> g k c", c=CC)
        PSL0 = pc.tile([128, 5 * FS[0][1]], f32)
        PSL1 = pc.tile([128, 3 * FS[1][1]], f32)

        def comb_gather(s):
            l, b, k = _J0[s]
            ca = _lvl_cols[l] + b * W[l]
            nc.gpsimd.indirect_dma_start(
                out=PWALL[:, ca * CC:(ca + W[l]) * CC], out_offset=None,
                in_=comb_flat,
                in_offset=bass.IndirectOffsetOnAxis(ap=RIC[:, s:s + 1], axis=1))

        for s in range(5):              # level-0 row blocks
            comb_gather(s)
        nc.gpsimd.indirect_copy(XSYS[:, 0:6 * NW], SVVR[:], XSYS_IDX_B[:], True)
        nc.gpsimd.indirect_dma_start(
            out=PSL0[:, 0:576], out_offset=None, in_=clsT_flat,
            in_offset=bass.IndirectOffsetOnAxis(ap=PSIDX, axis=1))
        nc.gpsimd.indirect_dma_start(
            out=PSL1[:, 0:144], out_offset=None, in_=clsT_flat,
            in_offset=bass.IndirectOffsetOnAxis(ap=PSIDX1, axis=1))
        for s in range(5, 12):          # levels 1-4 row blocks
            comb_gather(s)

        XS74 = XSYS[:, 0 * NW:1 * NW]
        YS74 = XSYS[:, 1 * NW:2 * NW]
        X1Y1V = XSYS[:, 2 * NW:4 * NW]
        X2Y2V = XSYS[:, 4 * NW:6 * NW]

        # ---- small per-box tensors (not gather-gated; keep them out of the
        # critical index-chain window via a scheduler wait gate) --------------
        LBL = pc.tile([128, 1], f32)
        nc.vector.tensor_scalar(LBL[:], GT[:, 4:5], 0.0, float(C - 1), OP.max, OP.min)
        ONEHOT = pc.tile([128, C], f32)
        nc.vector.tensor_tensor(out=ONEHOT[:], in0=CCONST,
                                in1=LBL[:, 0:1].to_broadcast([128, C]),
                                op=OP.is_equal)
        VR3 = VR.rearrange("g (l j) -> g l j", j=4)
        x1v, y1v, x2v, y2v = VR3[:, :, 0], VR3[:, :, 1], VR3[:, :, 2], VR3[:, :, 3]
        EX = pc.tile([128, NLEV], f32)
        nc.vector.tensor_tensor(out=EX[:], in0=x1v, in1=x2v, op=OP.is_equal)
        EY = pc.tile([128, NLEV], f32)
        nc.vector.tensor_tensor(out=EY[:], in0=y1v, in1=y2v, op=OP.is_equal)
        EMX = pc.tile([128, NLEV], f32)
        nc.vector.tensor_tensor(out=EMX[:], in0=EX[:], in1=EY[:], op=OP.max)
        DX = pc.tile([128, NLEV], f32)
        nc.vector.tensor_tensor(out=DX[:], in0=x2v, in1=x1v, op=OP.subtract)
        DY = pc.tile([128, NLEV], f32)
        nc.vector.tensor_tensor(out=DY[:], in0=y2v, in1=y1v, op=OP.subtract)
        DN = pc.tile([128, NLEV], f32)
        nc.vector.tensor_tensor(out=DN[:], in0=DX[:], in1=DY[:], op=OP.mult)
        nc.vector.tensor_scalar(DN[:], DN[:], 1.0, None, OP.max)
        RECDN = pc.tile([128, NLEV], f32)
        nc.vector.reciprocal(RECDN[:], DN[:])
        SABS = pc.tile([128, 1], f32)
        nc.vector.tensor_reduce(SABS[:], GT[:, 0:4], axis=AX.X, op=OP.add,
                                apply_absolute_value=True)
        NV = pc.tile([128, 1], i32)
        nc.vector.tensor_scalar(NV[:], SABS[:], 0.0, None, OP.is_le)

        # ---- window mask + iou targets (need XSYS big block) ----------------
        XJROWY = pc.tile([128, 2 * NW], f32)
        nc.vector.tensor_tensor(out=XJROWY[:, 0:NW], in0=XS74, in1=J74, op=OP.add)
        nc.vector.tensor_tensor(out=XJROWY[:, NW:2 * NW], in0=YS74, in1=R74,
                                op=OP.add)
        MGE = pc.tile([128, 2 * NW], f32)
        nc.vector.tensor_tensor(out=MGE[:], in0=XJROWY[:], in1=X1Y1V, op=OP.is_ge)
        MLT = pc.tile([128, 2 * NW], f32)
        nc.vector.tensor_tensor(out=MLT[:], in0=XJROWY[:], in1=X2Y2V, op=OP.is_lt)
        nc.vector.tensor_tensor(out=MGE[:], in0=MGE[:], in1=MLT[:], op=OP.mult)
        M74 = pc.tile([128, NW], f32)
        nc.vector.tensor_tensor(out=M74[:], in0=MGE[:, 0:NW],
                                in1=MGE[:, NW:2 * NW], op=OP.mult)
        SXY2 = pc.tile([128, 2 * NW], f32)
        nc.vector.tensor_scalar(SXY2[:], XJROWY[:], 0.25, 0.125, OP.mult, OP.add)
        EDGEQ = pc.tile([128, 4 * NW], f32)
        nc.vector.tensor_tensor(
            out=EDGEQ[:].rearrange("g (q k) -> g q k", k=NW),
            in0=GT[:, 0:4, None].to_broadcast([128, 4, NW]),
            in1=INV4S296.rearrange("g (q k) -> g q k", k=NW),
            op=OP.mult)
        TLRB = pc.tile([128, 4 * NW], f32)
        nc.vector.tensor_tensor(out=TLRB[:, 0:2 * NW], in0=SXY2[:],
                                in1=EDGEQ[:, 0:2 * NW], op=OP.subtract)
        nc.vector.tensor_tensor(out=TLRB[:, 2 * NW:4 * NW],
                                in0=EDGEQ[:, 2 * NW:4 * NW],
                                in1=SXY2[:], op=OP.subtract)
        nc.vector.tensor_scalar(TLRB[:], TLRB[:], 0.0, None, OP.max)
        TSUM = pc.tile([128, 2 * NW], f32)
        nc.vector.tensor_tensor(out=TSUM[:], in0=TLRB[:, 0:2 * NW],
                                in1=TLRB[:, 2 * NW:4 * NW], op=OP.add)
        TAREA = pc.tile([128, NW], f32)
        nc.vector.tensor_tensor(out=TAREA[:], in0=TSUM[:, 0:NW],
                                in1=TSUM[:, NW:2 * NW], op=OP.mult)

        # ---- per-level pipelined compute ------------------------------------
        T1 = pc.tile([128, NF0], f32)
        T2 = pc.tile([128, NF0], f32)
        F0W = pc.tile([128, NW], f32)
        PSEL = pc.tile([128, NW], f32)
        P3 = pc.tile([128, 25 * C], f32)
        LN1 = pc.tile([128, NW], f32)
        LNP = pc.tile([128, NW], f32)
        SQ = pc.tile([128, NW], f32)
        SQ1 = pc.tile([128, NW], f32)
        CONTR = pc.tile([128, NW], f32)
        P4T = pc.tile([128, 4 * NW], f32)
        P4Tv = P4T[:].rearrange("g (c k) -> g c k", c=4)
        P4T3 = P4T[:].rearrange("g (q k) -> g q k", k=NW)
        TLRB3 = TLRB[:].rearrange("g (q k) -> g q k", k=NW)
        PS = pc.tile([128, 2 * NW], f32)
        PS3 = PS[:].rearrange("g (q k) -> g q k", k=NW)
        PAREA = pc.tile([128, NW], f32)
        MIN4 = pc.tile([128, 4 * NW], f32)
        MIN43 = MIN4[:].rearrange("g (q k) -> g q k", k=NW)
        WIHI = pc.tile([128, 2 * NW], f32)
        WIHI3 = WIHI[:].rearrange("g (q k) -> g q k", k=NW)
        AI = pc.tile([128, NW], f32)
        AU = pc.tile([128, NW], f32)
        LNAI = pc.tile([128, NW], f32)
        LNAU = pc.tile([128, NW], f32)
        LNR = pc.tile([128, NW], f32)
        FCM = pc.tile([128, NW], f32)
        LNM = pc.tile([128, NW], f32)
        TOT = pc.tile([128, NW], f32)
        SL5 = pc.tile([128, NLEV], f32)
        PSL0v = PSL0[:].rearrange("g (b x) -> g b x", x=FS[0][1])

        def f0_region(ca, cb, eng_mult=None):
            pv = PWv[:, ca:cb, 0:C]
            nc.scalar.activation(
                T1[:, ca * C:cb * C].rearrange("g (k c) -> g k c", c=C),
                pv, AF.Ln, bias=1.0, scale=-1.0)
            nc.scalar.activation(
                T2[:, ca * C:cb * C].rearrange("g (k c) -> g k c", c=C),
                pv, AF.Square)
            (eng_mult or nc.vector).tensor_tensor(
                out=T2[:, ca * C:cb * C], in0=T2[:, ca * C:cb * C],
                in1=T1[:, ca * C:cb * C], op=OP.mult)
            nc.vector.tensor_reduce(
                F0W[:, ca:cb],
                T2[:, ca * C:cb * C].rearrange("g (k c) -> g k c", c=C),
                axis=AX.X, op=OP.add)

        def psel_region(l):
            a, b = _lvl_cols[l], _lvl_cols[l + 1]
            if l == 0:
                nc.scalar.copy(
                    PSEL[:, a:b].rearrange("g (b j) -> g b j", j=W[0]),
                    PSL0v[:, :, 0:W[0]])
            elif l == 1:
                nc.scalar.copy(
                    PSEL[:, a:b].rearrange("g (b j) -> g b j", j=W[1]),
                    PSL1[:].rearrange("g (b x) -> g b x", x=FS[1][1])[:, :, 0:W[1]])
            else:
                a3, b3 = a - _lvl_cols[1], b - _lvl_cols[1]
                nc.gpsimd.tensor_tensor(
                    out=P3[:, a3 * C:b3 * C].rearrange("g (k c) -> g k c", c=C),
                    in0=PWv[:, a:b, 0:C],
                    in1=ONEHOT[:, None, :].to_broadcast([128, b - a, C]),
                    op=OP.mult)
                nc.vector.tensor_reduce(
                    PSEL[:, a:b],
                    P3[:, a3 * C:b3 * C].rearrange("g (k c) -> g k c", c=C),
                    axis=AX.X, op=OP.add)

        def contr_region(a, b):
            nc.scalar.activation(LN1[:, a:b], PSEL[:, a:b], AF.Ln,
                                 bias=1.0, scale=-1.0)
            nc.scalar.activation(LNP[:, a:b], PSEL[:, a:b], AF.Ln)
            nc.scalar.activation(SQ[:, a:b], PSEL[:, a:b], AF.Square)
            nc.scalar.activation(SQ1[:, a:b], PSEL[:, a:b], AF.Square,
                                 bias=1.0, scale=-1.0)
            nc.vector.tensor_tensor(out=SQ1[:, a:b], in0=SQ1[:, a:b],
                                    in1=LNP[:, a:b], op=OP.mult)
            nc.vector.tensor_tensor(out=SQ[:, a:b], in0=SQ[:, a:b],
                                    in1=LN1[:, a:b], op=OP.mult)
            nc.vector.scalar_tensor_tensor(CONTR[:, a:b], SQ1[:, a:b], 1.0 / 3.0,
                                           SQ[:, a:b], OP.mult, OP.subtract)

        def p4t_region(l, eng):
            a, b = _lvl_cols[l], _lvl_cols[l + 1]
            eng.tensor_copy(
                P4Tv[:, :, a:b],
                PWv[:, a:b, C:CC].rearrange("g k c -> g c k"))

        def iou_region(a, b, eng):
            eng.tensor_tensor(out=PS3[:, :, a:b], in0=P4T3[:, 0:2, a:b],
                              in1=P4T3[:, 2:4, a:b], op=OP.add)
            eng.tensor_tensor(out=PAREA[:, a:b], in0=PS3[:, 0, a:b],
                              in1=PS3[:, 1, a:b], op=OP.mult)
            nc.vector.tensor_tensor(out=MIN43[:, :, a:b], in0=P4T3[:, :, a:b],
                                    in1=TLRB3[:, :, a:b], op=OP.min)
            eng.tensor_tensor(out=WIHI3[:, :, a:b], in0=MIN43[:, 0:2, a:b],
                              in1=MIN43[:, 2:4, a:b], op=OP.add)
            eng.tensor_tensor(out=AI[:, a:b], in0=WIHI3[:, 0, a:b],
                              in1=WIHI3[:, 1, a:b], op=OP.mult)
            eng.tensor_tensor(out=AU[:, a:b], in0=TAREA[:, a:b],
                              in1=PAREA[:, a:b], op=OP.add)
            eng.tensor_tensor(out=AU[:, a:b], in0=AU[:, a:b],
                              in1=AI[:, a:b], op=OP.subtract)
            nc.scalar.activation(LNAI[:, a:b], AI[:, a:b], AF.Ln, bias=CONSTEPS)
            nc.scalar.activation(LNAU[:, a:b], AU[:, a:b], AF.Ln, bias=CONSTEPS)
            eng.tensor_tensor(out=LNR[:, a:b], in0=LNAI[:, a:b],
                              in1=LNAU[:, a:b], op=OP.subtract)

        def tot_region(a, b):
            nc.vector.tensor_tensor(out=FCM[:, a:b], in0=F0W[:, a:b],
                                    in1=CONTR[:, a:b], op=OP.add)
            nc.vector.tensor_tensor(out=FCM[:, a:b], in0=FCM[:, a:b],
                                    in1=M74[:, a:b], op=OP.mult)
            nc.vector.tensor_tensor(out=LNM[:, a:b], in0=LNR[:, a:b],
                                    in1=M74[:, a:b], op=OP.mult)
            nc.vector.scalar_tensor_tensor(TOT[:, a:b], FCM[:, a:b], 0.75,
                                           LNM[:, a:b], OP.mult, OP.add)

        def sl_region(l):
            a, b = _lvl_cols[l], _lvl_cols[l + 1]
            nc.vector.tensor_reduce(SL5[:, l:l + 1], TOT[:, a:b],
                                    axis=AX.X, op=OP.add)

        L0A, L0B = _lvl_cols[0], _lvl_cols[1]
        L4B = _lvl_cols[5]
        # level-0 f0 per row-block (pipelines with its 5 gathers); psel-L1
        # one-hot issued early so the psel->contr->tot chain isn't the tail
        f0_region(0 * W[0], 1 * W[0])
        f0_region(1 * W[0], 2 * W[0])
        psel_region(0)
        contr_region(L0A, L0B)
        p4t_region(0, nc.vector)
        iou_region(L0A, L0B, nc.vector)
        for b in range(2, NH[0]):
            f0_region(b * W[0], (b + 1) * W[0])
        tot_region(L0A, L0B)
        sl_region(0)
        psel_region(1)
        contr_region(_lvl_cols[1], _lvl_cols[2])
        f0_region(_lvl_cols[1], _lvl_cols[2])
        for l in range(2, NLEV):
            psel_region(l)
            f0_region(_lvl_cols[l], _lvl_cols[l + 1], eng_mult=nc.gpsimd)
        contr_region(_lvl_cols[2], L4B)
        for l in range(1, NLEV):
            p4t_region(l, nc.gpsimd)
        iou_region(L0B, L4B, nc.vector)
        tot_region(L0B, L4B)
        for l in range(1, NLEV):
            sl_region(l)

        # ---- combine halves, finalize loss, argmin --------------------------
        LVA = pc.tile([128, NLEV], f32)
        nc.vector.tensor_tensor(out=LVA[:], in0=SL5[:], in1=RECDN[:], op=OP.mult)
        SLH = pc.tile([64, NLEV], f32)
        nc.vector.stream_shuffle(SLH[:], LVA[64:128, :], list(range(32)))
        LOSSH0 = pc.tile([64, NLEV], f32)
        nc.vector.scalar_tensor_tensor(LOSSH0[:], EMX[0:64, :], BIG,
                                       LVA[0:64, :], OP.mult, OP.subtract)
        LOSS = pc.tile([64, NLEV], f32)
        nc.vector.tensor_tensor(out=LOSS[:], in0=LOSSH0[:], in1=SLH[:],
                                op=OP.subtract)
        MBEST = pc.tile([64, 1], f32)
        nc.vector.tensor_reduce(MBEST[:], LOSS[:], axis=AX.X, op=OP.min)
        EQ5 = pc.tile([64, NLEV], f32)
        nc.vector.tensor_tensor(out=EQ5[:], in0=LOSS[:],
                                in1=MBEST[:, 0:1].to_broadcast([64, NLEV]),
                                op=OP.is_equal)
        PEN5 = pc.tile([64, NLEV], f32)
        nc.vector.scalar_tensor_tensor(PEN5[:], EQ5[:], -BIG2, CLB5[0:64, :],
                                       OP.mult, OP.add)
        IDX = pc.tile([64, 1], f32)
        nc.vector.tensor_reduce(IDX[:], PEN5[:], axis=AX.X, op=OP.min)
        nc.vector.copy_predicated(IDX[:], NV[0:64, :], CONSTM1[0:64, :])
        IDXI = pc.tile([64, 1], i32)
        nc.vector.tensor_copy(IDXI[:], IDX[:])
        nc.sync.dma_start(out_lvl.ap()[:, None], IDXI[:])

    nc.compile()
    return nc


_NC_CACHE = None


def _get_nc():
    global _NC_CACHE
    if _NC_CACHE is None:
        _NC_CACHE = build_nc(num_devices=8)
    return _NC_CACHE


def _host_idx(gt):
    """Per-image gather descriptors (pure [64,5] geometry metadata):
    row-block start offsets into the combined array + the psel span start."""
    boxes = gt[:, 0:4].astype(np.float64)
    label = np.clip(gt[:, 4].astype(np.int64), 0, C - 1)
    cx = (boxes[:, 0] + boxes[:, 2]) * 0.5
    cy = (boxes[:, 1] + boxes[:, 3]) * 0.5
    hw = (boxes[:, 2] - boxes[:, 0]) * 0.5 * 0.2
    hh = (boxes[:, 3] - boxes[:, 1]) * 0.5 * 0.2
    # float32 arithmetic to match the device V-chain exactly
    qx1 = (np.float32(0.6) * boxes[:, 0].astype(np.float32)
           + np.float32(0.4) * boxes[:, 2].astype(np.float32))
    qy1 = (np.float32(0.6) * boxes[:, 1].astype(np.float32)
           + np.float32(0.4) * boxes[:, 3].astype(np.float32))
    out = np.zeros((128, 14), np.int32)
    for s, (l, b, k) in enumerate(_J0):
        fh, fw = FS[l]
        w = W[l]
        v1x = (qx1 * np.float32(1.0 / STRIDES[l])).astype(np.float32)
        v1y = (qy1 * np.float32(1.0 / STRIDES[l])).astype(np.float32)
        xs = np.clip(np.floor(v1x), 0, fw - 1)
        ys = np.clip(np.floor(v1y), 0, fh - 1)
        xs = np.minimum(xs, fw - w)
        ys_lo = np.minimum(ys + b, fh - 1)
        ys_hi = np.minimum(ys + NH[l] + b, fh - 1)
        out[0:64, s] = ((ys_lo * fw + xs + ROWOFS[l]) * CC).astype(np.int32)
        out[64:128, s] = ((ys_hi * fw + xs + ROWOFS[l]) * CC).astype(np.int32)
    fh, fw = FS[0]
    v1x = (qx1 * np.float32(1.0 / STRIDES[0])).astype(np.float32)
    v1y = (qy1 * np.float32(1.0 / STRIDES[0])).astype(np.float32)
    xs = np.minimum(np.clip(np.floor(v1x), 0, fw - 1), fw - W[0])
    ys = np.minimum(np.clip(np.floor(v1y), 0, fh - 1), fh - W[0])
    st = label * NLOC + (ys * fw + xs).astype(np.int64)
    out[0:64, 12] = st.astype(np.int32)
    out[64:128, 12] = (st + NH[0] * fw).astype(np.int32)
    fh, fw = FS[1]
    v1x = (qx1 * np.float32(1.0 / STRIDES[1])).astype(np.float32)
    v1y = (qy1 * np.float32(1.0 / STRIDES[1])).astype(np.float32)
    xs = np.minimum(np.clip(np.floor(v1x), 0, fw - 1), fw - W[1])
    ys = np.minimum(np.clip(np.floor(v1y), 0, fh - 1), fh - W[1])
    st = label * NLOC + ROWOFS[1] + (ys * fw + xs).astype(np.int64)
    out[0:64, 13] = st.astype(np.int32)
    out[64:128, 13] = (st + NH[1] * fw).astype(np.int32)
    return out


def kernel(cls_pred, regr_pred, feature_shapes, gt_boxes):
    from concourse.bass_utils import run_bass_kernel_spmd

    B = cls_pred.shape[0]
    assert B == 8 and cls_pred.shape[1] == NLOC and cls_pred.shape[2] == C
    nc = _get_nc()
    cls_pred = np.asarray(cls_pred, dtype=np.float32)
    regr_pred = np.asarray(regr_pred, dtype=np.float32)
    gt_boxes = np.asarray(gt_boxes, dtype=np.float32)
    in_maps = [
        {
            "comb_b": np.ascontiguousarray(
                np.concatenate([cls_pred[b], regr_pred[b]], axis=-1)),
            "clsT_b": np.ascontiguousarray(cls_pred[b].T),
            "gt_b": np.ascontiguousarray(gt_boxes[b]),
            "idx_b": _host_idx(gt_boxes[b]),
        }
        for b in range(B)
    ]
    res = run_bass_kernel_spmd(nc, in_maps, list(range(B)))
    out = np.stack([np.asarray(res.results[b]["out_lvl"]).reshape(G)
                    for b in range(B)])
    return out.reshape(-1).astype(np.int32)


# revision 16
# speedup vs baseline: 1.0091x; 1.0091x over previous
"""Trainium2 Bass kernel for nn_MetaSelectTarget (FPN level assignment).

Strategy (v3):
  - Data-parallel over batch: B=8 images -> 8 NeuronCores, one image each.
  - 128 partitions: box g lives on partitions g and g+64, each half handling
    half of the window rows.  Unified column layout packs all levels' window
    cells into 70 slots.
  - Host-side layout prep (pure data movement): cls and regr are concatenated
    into one [NLOC, 84] array so the 12 row-block gathers fetch class AND
    regr data together; cls is also transposed to [80, NLOC] so the level-0
    per-gt-class probabilities come from ONE span gather (start =
    label*NLOC + y1*fw + x1, static in-span offsets b*fw+j).
  - psel for levels 1-4 via one-hot multiply + class reduce.
  - Everything after the gathers is split per level so compute pipelines
    behind the serial SWDGE descriptor generation; only level-4's small
    tail trails the last gather.
  - Tail: halves combined with a DVE stream_shuffle (partition crossbar).
"""

import numpy as np

import concourse.bass as bass
import concourse.bacc as bacc
import concourse.tile as tile
from concourse import mybir
from contextlib import ExitStack

f32 = mybir.dt.float32
i32 = mybir.dt.int32
u16 = mybir.dt.uint16
AF = mybir.ActivationFunctionType
OP = mybir.AluOpType
AX = mybir.AxisListType

G = 64
C = 80
CC = 84                   # combined row: 80 cls + 4 regr
FS = [(128, 128), (64, 64), (32, 32), (16, 16), (8, 8)]
STRIDES = [8.0, 16.0, 32.0, 64.0, 128.0]
ROWOFS = [0, 16384, 20480, 21504, 21760]
NLOC = 21824
W = [9, 5, 3, 2, 2]
NH = [5, 3, 2, 1, 1]
EPS = 1e-7
BIG = 1e7
BIG2 = 16.0
NLEV = 5

_cols = []      # (level, block, slot)
for _l in range(NLEV):
    for _b in range(NH[_l]):
        for _j in range(W[_l]):
            _cols.append((_l, _b, _j))
NW = len(_cols)           # 70
assert NW == 70
_lvl_cols = [sum(NH[l] * W[l] for l in range(lv)) for lv in range(NLEV + 1)]
_J0 = [(l, b, k) for k, (l, b, j) in enumerate(_cols) if j == 0]
assert len(_J0) == 12


def _wrap_idx(mapv):
    n = len(mapv)
    s = (n + 15) // 16
    a = np.zeros((128, s), np.uint16)
    for i, m in enumerate(mapv):
        a[np.arange(8) * 16 + i % 16, i // 16] = m
    return a


def build_nc(num_devices=8):
    nc = bacc.Bacc("TRN2", target_bir_lowering=False, num_devices=num_devices)

    comb_b = nc.dram_tensor("comb_b", [NLOC, CC], f32, kind="ExternalInput")
    clsT_b = nc.dram_tensor("clsT_b", [C, NLOC], f32, kind="ExternalInput")
    gt_b = nc.dram_tensor("gt_b", [G, 5], f32, kind="ExternalInput")
    idx_b = nc.dram_tensor("idx_b", [128, 14], i32, kind="ExternalInput")
    out_lvl = nc.dram_tensor("out_lvl", [G], i32, kind="ExternalOutput")

    comb_flat = comb_b.ap().rearrange("n c -> (n c)")[None, :]
    clsT_flat = clsT_b.ap().rearrange("n c -> (n c)")[None, :]

    # ---- inline constants --------------------------------------------------
    recip = np.zeros((128, NLEV, 4), np.float32)
    maskF = np.zeros((128, NLEV, 4), np.float32)
    maskC = np.zeros((128, NLEV, 4), np.float32)
    clo = np.zeros((128, NLEV, 4), np.float32)
    chi = np.zeros((128, NLEV, 4), np.float32)
    shi = np.zeros((128, NLEV, 4), np.float32)
    for l in range(NLEV):
        fh, fw = FS[l]
        w = W[l]
        recip[:, l, :] = 1.0 / STRIDES[l]
        maskF[:, l, 0] = maskF[:, l, 1] = 1.0
        maskC[:, l, 2] = maskC[:, l, 3] = 1.0
        clo[:, l, :] = [0.0, 0.0, 1.0, 1.0]
        chi[:, l, :] = [fw - 1, fh - 1, fw, fh]
        shi[:, l, :] = [fw - w, fh - w, 1e9, 1e9]

    r74 = np.zeros((128, NW), np.float32)
    j74 = np.zeros((128, NW), np.float32)
    inv4s296 = np.zeros((128, 4 * NW), np.float32)
    for k, (l, b, j) in enumerate(_cols):
        r74[0:64, k] = b
        r74[64:128, k] = NH[l] + b
        j74[:, k] = j
        for ch in range(4):
            inv4s296[:, ch * NW + k] = 1.0 / (4.0 * STRIDES[l])

    r12 = np.zeros((128, 12), np.float32)
    fw12 = np.zeros((128, 12), np.float32)
    fhm112 = np.zeros((128, 12), np.float32)
    rofs12 = np.zeros((128, 12), np.float32)
    for s, (l, b, k) in enumerate(_J0):
        r12[0:64, s] = b
        r12[64:128, s] = NH[l] + b
        fw12[:, s] = FS[l][1]
        fhm112[:, s] = FS[l][0] - 1
        rofs12[:, s] = ROWOFS[l]

    cconst = np.tile(np.arange(C, dtype=np.float32), (128, 1))
    clb5 = np.tile(np.arange(NLEV, dtype=np.float32) + BIG2, (128, 1))
    constm1 = np.full((128, 1), -1.0, np.float32)
    consteps = np.full((128, 1), EPS, np.float32)
    half640 = np.zeros((128, 1), np.float32)
    half640[64:128, 0] = float(NH[0] * FS[0][1])

    consts = np.concatenate(
        [recip.reshape(128, -1), maskF.reshape(128, -1), maskC.reshape(128, -1),
         clo.reshape(128, -1), chi.reshape(128, -1), shi.reshape(128, -1),
         r74, j74, inv4s296,
         r12, fw12, fhm112, rofs12,
         cconst, clb5, constm1, consteps, half640], axis=1)
    t_consts = nc.inline_tensor(consts, "c_all")
    NCONST = consts.shape[1]

    xsys_map = []
    for q in range(6):
        for k, (l, b, j) in enumerate(_cols):
            xsys_map.append([4 * l + 0, 4 * l + 1, 20 + 4 * l + 0, 20 + 4 * l + 1,
                             20 + 4 * l + 2, 20 + 4 * l + 3][q])
    for s, (l, b, k) in enumerate(_J0):
        xsys_map.append(4 * l + 0)
    for s, (l, b, k) in enumerate(_J0):
        xsys_map.append(4 * l + 1)
    t_xsys_idx_s = nc.inline_tensor(_wrap_idx(xsys_map[6 * NW:]), "xsys_idx_s")
    t_xsys_idx_b = nc.inline_tensor(_wrap_idx(xsys_map[:6 * NW]), "xsys_idx_b")
    NXS = len(xsys_map)  # 444

    NPW = NW * CC
    NF0 = NW * C

    with tile.TileContext(nc) as tc, ExitStack() as ctx:
        pc = ctx.enter_context(tc.tile_pool(name="pc", bufs=1))

        # host-computed gather descriptors first: they gate the gathers
        IDXT = pc.tile([128, 14], i32)
        nc.sync.dma_start(IDXT[:], idx_b[:])
        GT = pc.tile([128, 5], f32)
        nc.sync.dma_start(GT[0:64, :], gt_b[:])
        nc.gpsimd.dma_start(GT[64:128, :], gt_b[:])
        CST = pc.tile([128, NCONST], f32)
        nc.sync.dma_start(CST[:, 0:120], t_consts[:, 0:120])
        nc.sync.dma_start(CST[:, 120:NCONST], t_consts[:, 120:NCONST])
        XSYS_IDX_S = pc.tile([128, 2], u16)
        XSYS_IDX_B = pc.tile([128, (6 * NW + 15) // 16], u16)
        nc.sync.dma_start(XSYS_IDX_S[:], t_xsys_idx_s[:])
        nc.sync.dma_start(XSYS_IDX_B[:], t_xsys_idx_b[:])
        WARM = pc.tile([1, 2], f32)
        nc.vector.memset(WARM[:], 0.5)
        nc.scalar.activation(WARM[:, 0:1], WARM[:, 0:1], AF.Ln)
        nc.scalar.activation(WARM[:, 1:2], WARM[:, 1:2], AF.Square)

        off = 0
        def _cview(n):
            nonlocal off
            v = CST[:, off:off + n]
            off += n
            return v
        RECIP = _cview(NLEV * 4)
        MASKF = _cview(NLEV * 4)
        MASKC = _cview(NLEV * 4)
        CLO = _cview(NLEV * 4)
        CHI = _cview(NLEV * 4)
        SHI = _cview(NLEV * 4)
        R74 = _cview(NW)
        J74 = _cview(NW)
        INV4S296 = _cview(4 * NW)
        R12 = _cview(12)
        FW12 = _cview(12)
        FHM112 = _cview(12)
        ROFS12 = _cview(12)
        CCONST = _cview(C)
        CLB5 = _cview(NLEV)
        CONSTM1 = _cview(1)
        CONSTEPS = _cview(1)
        HALF640 = _cview(1)

        # ---- box math: critical path to the gathers -------------------------
        GTSW = pc.tile([128, 4], f32)
        nc.vector.tensor_copy(GTSW[:, 0:2], GT[:, 2:4])
        nc.vector.tensor_copy(GTSW[:, 2:4], GT[:, 0:2])
        Q = pc.tile([128, 4], f32)
        nc.vector.tensor_scalar(Q[:], GTSW[:], 0.4, None, OP.mult)
        nc.vector.scalar_tensor_tensor(Q[:], GT[:, 0:4], 0.6, Q[:], OP.mult, OP.add)

        SVVR = pc.tile([128, 40], f32)
        SV = SVVR[:, 0:20]
        VR = SVVR[:, 20:40]
        V = pc.tile([128, NLEV * 4], f32)
        nc.vector.tensor_tensor(
            out=V[:].rearrange("g (l j) -> g l j", j=4),
            in0=Q[:, None, :].to_broadcast([128, NLEV, 4]),
            in1=RECIP.rearrange("g (l j) -> g l j", j=4),
            op=OP.mult,
        )
        VI = pc.tile([128, NLEV * 4], i32)
        nc.vector.tensor_copy(VI[:], V[:])
        VF = pc.tile([128, NLEV * 4], f32)
        nc.vector.tensor_copy(VF[:], VI[:])
        GG = pc.tile([128, NLEV * 4], f32)
        nc.vector.tensor_tensor(out=GG[:], in0=VF[:], in1=V[:], op=OP.is_gt)
        LL = pc.tile([128, NLEV * 4], f32)
        nc.vector.tensor_tensor(out=LL[:], in0=VF[:], in1=V[:], op=OP.is_lt)
        nc.vector.tensor_tensor(out=GG[:], in0=GG[:], in1=MASKF, op=OP.mult)
        nc.vector.tensor_tensor(out=LL[:], in0=LL[:], in1=MASKC, op=OP.mult)
        nc.vector.tensor_tensor(out=VR, in0=VF[:], in1=GG[:], op=OP.subtract)
        nc.vector.tensor_tensor(out=VR, in0=VR, in1=LL[:], op=OP.add)
        nc.vector.tensor_tensor(out=VR, in0=VR, in1=CLO, op=OP.max)
        nc.vector.tensor_tensor(out=VR, in0=VR, in1=CHI, op=OP.min)
        nc.vector.tensor_tensor(out=SV, in0=VR, in1=SHI, op=OP.min)

        XSYS = pc.tile([128, NXS], f32)
        nc.gpsimd.indirect_copy(XSYS[:, 6 * NW:NXS], SVVR[:], XSYS_IDX_S[:], True)
        RIC = IDXT[:, 0:12]
        PSIDX = IDXT[:, 12:13]
        PSIDX1 = IDXT[:, 13:14]

        # ---- gathers: L0 blocks first, then psel span + big broadcast, then
        # levels 1-4 (order == compute order so everything pipelines) ---------
        PWALL = pc.tile([128, NPW], f32)
        PWv = PWALL[:].rearrange("g (k c) -> g k c", c=CC)
        PSL0 = pc.tile([128, 5 * FS[0][1]], f32)
        PSL1 = pc.tile([128, 3 * FS[1][1]], f32)

        def comb_gather(s):
            l, b, k = _J0[s]
            ca = _lvl_cols[l] + b * W[l]
            nc.gpsimd.indirect_dma_start(
                out=PWALL[:, ca * CC:(ca + W[l]) * CC], out_offset=None,
                in_=comb_flat,
                in_offset=bass.IndirectOffsetOnAxis(ap=RIC[:, s:s + 1], axis=1))

        for s in range(5):              # level-0 row blocks
            comb_gather(s)
        nc.gpsimd.indirect_copy(XSYS[:, 0:6 * NW], SVVR[:], XSYS_IDX_B[:], True)
        nc.gpsimd.indirect_dma_start(
            out=PSL0[:, 0:576], out_offset=None, in_=clsT_flat,
            in_offset=bass.IndirectOffsetOnAxis(ap=PSIDX, axis=1))
        nc.gpsimd.indirect_dma_start(
            out=PSL1[:, 0:144], out_offset=None, in_=clsT_flat,
            in_offset=bass.IndirectOffsetOnAxis(ap=PSIDX1, axis=1))
        for s in range(5, 12):          # levels 1-4 row blocks
            comb_gather(s)

        XS74 = XSYS[:, 0 * NW:1 * NW]
        YS74 = XSYS[:, 1 * NW:2 * NW]
        X1Y1V = XSYS[:, 2 * NW:4 * NW]
        X2Y2V = XSYS[:, 4 * NW:6 * NW]

        # ---- small per-box tensors (not gather-gated; keep them out of the
        # critical index-chain window via a scheduler wait gate) --------------
        LBL = pc.tile([128, 1], f32)
        nc.vector.tensor_scalar(LBL[:], GT[:, 4:5], 0.0, float(C - 1), OP.max, OP.min)
        ONEHOT = pc.tile([128, C], f32)
        nc.vector.tensor_tensor(out=ONEHOT[:], in0=CCONST,
                                in1=LBL[:, 0:1].to_broadcast([128, C]),
                                op=OP.is_equal)
        VR3 = VR.rearrange("g (l j) -> g l j", j=4)
        x1v, y1v, x2v, y2v = VR3[:, :, 0], VR3[:, :, 1], VR3[:, :, 2], VR3[:, :, 3]
        EX = pc.tile([128, NLEV], f32)
        nc.vector.tensor_tensor(out=EX[:], in0=x1v, in1=x2v, op=OP.is_equal)
        EY = pc.tile([128, NLEV], f32)
        nc.vector.tensor_tensor(out=EY[:], in0=y1v, in1=y2v, op=OP.is_equal)
        EMX = pc.tile([128, NLEV], f32)
        nc.vector.tensor_tensor(out=EMX[:], in0=EX[:], in1=EY[:], op=OP.max)
        DX = pc.tile([128, NLEV], f32)
        nc.vector.tensor_tensor(out=DX[:], in0=x2v, in1=x1v, op=OP.subtract)
        DY = pc.tile([128, NLEV], f32)
        nc.vector.tensor_tensor(out=DY[:], in0=y2v, in1=y1v, op=OP.subtract)
        DN = pc.tile([128, NLEV], f32)
        nc.vector.tensor_tensor(out=DN[:], in0=DX[:], in1=DY[:], op=OP.mult)
        nc.vector.tensor_scalar(DN[:], DN[:], 1.0, None, OP.max)
        RECDN = pc.tile([128, NLEV], f32)
        nc.vector.reciprocal(RECDN[:], DN[:])
        SABS = pc.tile([128, 1], f32)
        nc.vector.tensor_reduce(SABS[:], GT[:, 0:4], axis=AX.X, op=OP.add,
                                apply_absolute_value=True)
        NV = pc.tile([128, 1], i32)
        nc.vector.tensor_scalar(NV[:], SABS[:], 0.0, None, OP.is_le)

        # ---- window mask + iou targets (need XSYS big block) ----------------
        XJROWY = pc.tile([128, 2 * NW], f32)
        nc.vector.tensor_tensor(out=XJROWY[:, 0:NW], in0=XS74, in1=J74, op=OP.add)
        nc.vector.tensor_tensor(out=XJROWY[:, NW:2 * NW], in0=YS74, in1=R74,
                                op=OP.add)
        MGE = pc.tile([128, 2 * NW], f32)
        nc.vector.tensor_tensor(out=MGE[:], in0=XJROWY[:], in1=X1Y1V, op=OP.is_ge)
        MLT = pc.tile([128, 2 * NW], f32)
        nc.vector.tensor_tensor(out=MLT[:], in0=XJROWY[:], in1=X2Y2V, op=OP.is_lt)
        nc.vector.tensor_tensor(out=MGE[:], in0=MGE[:], in1=MLT[:], op=OP.mult)
        M74 = pc.tile([128, NW], f32)
        nc.vector.tensor_tensor(out=M74[:], in0=MGE[:, 0:NW],
                                in1=MGE[:, NW:2 * NW], op=OP.mult)
        SXY2 = pc.tile([128, 2 * NW], f32)
        nc.vector.tensor_scalar(SXY2[:], XJROWY[:], 0.25, 0.125, OP.mult, OP.add)
        EDGEQ = pc.tile([128, 4 * NW], f32)
        nc.vector.tensor_tensor(
            out=EDGEQ[:].rearrange("g (q k) -> g q k", k=NW),
            in0=GT[:, 0:4, None].to_broadcast([128, 4, NW]),
            in1=INV4S296.rearrange("g (q k) -> g q k", k=NW),
            op=OP.mult)
        TLRB = pc.tile([128, 4 * NW], f32)
        nc.vector.tensor_tensor(out=TLRB[:, 0:2 * NW], in0=SXY2[:],
                                in1=EDGEQ[:, 0:2 * NW], op=OP.subtract)
        nc.vector.tensor_tensor(out=TLRB[:, 2 * NW:4 * NW],
                                in0=EDGEQ[:, 2 * NW:4 * NW],
                                in1=SXY2[:], op=OP.subtract)
        nc.vector.tensor_scalar(TLRB[:], TLRB[:], 0.0, None, OP.max)
        TSUM = pc.tile([128, 2 * NW], f32)
        nc.vector.tensor_tensor(out=TSUM[:], in0=TLRB[:, 0:2 * NW],
                                in1=TLRB[:, 2 * NW:4 * NW], op=OP.add)
        TAREA = pc.tile([128, NW], f32)
        nc.vector.tensor_tensor(out=TAREA[:], in0=TSUM[:, 0:NW],
                                in1=TSUM[:, NW:2 * NW], op=OP.mult)

        # ---- per-level pipelined compute ------------------------------------
        T1 = pc.tile([128, NF0], f32)
        T2 = pc.tile([128, NF0], f32)
        F0W = pc.tile([128, NW], f32)
        PSEL = pc.tile([128, NW], f32)
        P3 = pc.tile([128, 25 * C], f32)
        LN1 = pc.tile([128, NW], f32)
        LNP = pc.tile([128, NW], f32)
        SQ = pc.tile([128, NW], f32)
        SQ1 = pc.tile([128, NW], f32)
        CONTR = pc.tile([128, NW], f32)
        P4T = pc.tile([128, 4 * NW], f32)
        P4Tv = P4T[:].rearrange("g (c k) -> g c k", c=4)
        P4T3 = P4T[:].rearrange("g (q k) -> g q k", k=NW)
        TLRB3 = TLRB[:].rearrange("g (q k) -> g q k", k=NW)
        PS = pc.tile([128, 2 * NW], f32)
        PS3 = PS[:].rearrange("g (q k) -> g q k", k=NW)
        PAREA = pc.tile([128, NW], f32)
        MIN4 = pc.tile([128, 4 * NW], f32)
        MIN43 = MIN4[:].rearrange("g (q k) -> g q k", k=NW)
        WIHI = pc.tile([128, 2 * NW], f32)
        WIHI3 = WIHI[:].rearrange("g (q k) -> g q k", k=NW)
        AI = pc.tile([128, NW], f32)
        AU = pc.tile([128, NW], f32)
        LNAI = pc.tile([128, NW], f32)
        LNAU = pc.tile([128, NW], f32)
        LNR = pc.tile([128, NW], f32)
        FCM = pc.tile([128, NW], f32)
        LNM = pc.tile([128, NW], f32)
        TOT = pc.tile([128, NW], f32)
        SL5 = pc.tile([128, NLEV], f32)
        PSL0v = PSL0[:].rearrange("g (b x) -> g b x", x=FS[0][1])

        def f0_region(ca, cb, eng_mult=None):
            pv = PWv[:, ca:cb, 0:C]
            nc.scalar.activation(
                T1[:, ca * C:cb * C].rearrange("g (k c) -> g k c", c=C),
                pv, AF.Ln, bias=1.0, scale=-1.0)
            nc.scalar.activation(
                T2[:, ca * C:cb * C].rearrange("g (k c) -> g k c", c=C),
                pv, AF.Square)
            (eng_mult or nc.vector).tensor_tensor(
                out=T2[:, ca * C:cb * C], in0=T2[:, ca * C:cb * C],
                in1=T1[:, ca * C:cb * C], op=OP.mult)
            nc.vector.tensor_reduce(
                F0W[:, ca:cb],
                T2[:, ca * C:cb * C].rearrange("g (k c) -> g k c", c=C),
                axis=AX.X, op=OP.add)

        def psel_region(l):
            a, b = _lvl_cols[l], _lvl_cols[l + 1]
            if l == 0:
                nc.scalar.copy(
                    PSEL[:, a:b].rearrange("g (b j) -> g b j", j=W[0]),
                    PSL0v[:, :, 0:W[0]])
            elif l == 1:
                nc.scalar.copy(
                    PSEL[:, a:b].rearrange("g (b j) -> g b j", j=W[1]),
                    PSL1[:].rearrange("g (b x) -> g b x", x=FS[1][1])[:, :, 0:W[1]])
            else:
                a3, b3 = a - _lvl_cols[1], b - _lvl_cols[1]
                nc.gpsimd.tensor_tensor(
                    out=P3[:, a3 * C:b3 * C].rearrange("g (k c) -> g k c", c=C),
                    in0=PWv[:, a:b, 0:C],
                    in1=ONEHOT[:, None, :].to_broadcast([128, b - a, C]),
                    op=OP.mult)
                nc.vector.tensor_reduce(
                    PSEL[:, a:b],
                    P3[:, a3 * C:b3 * C].rearrange("g (k c) -> g k c", c=C),
                    axis=AX.X, op=OP.add)

        def contr_region(a, b):
            nc.scalar.activation(LN1[:, a:b], PSEL[:, a:b], AF.Ln,
                                 bias=1.0, scale=-1.0)
            nc.scalar.activation(LNP[:, a:b], PSEL[:, a:b], AF.Ln)
            nc.scalar.activation(SQ[:, a:b], PSEL[:, a:b], AF.Square)
            nc.scalar.activation(SQ1[:, a:b], PSEL[:, a:b], AF.Square,
                                 bias=1.0, scale=-1.0)
            nc.vector.tensor_tensor(out=SQ1[:, a:b], in0=SQ1[:, a:b],
                                    in1=LNP[:, a:b], op=OP.mult)
            nc.vector.tensor_tensor(out=SQ[:, a:b], in0=SQ[:, a:b],
                                    in1=LN1[:, a:b], op=OP.mult)
            nc.vector.scalar_tensor_tensor(CONTR[:, a:b], SQ1[:, a:b], 1.0 / 3.0,
                                           SQ[:, a:b], OP.mult, OP.subtract)

        def p4t_region(l, eng):
            a, b = _lvl_cols[l], _lvl_cols[l + 1]
            eng.tensor_copy(
                P4Tv[:, :, a:b],
                PWv[:, a:b, C:CC].rearrange("g k c -> g c k"))

        def iou_region(a, b, eng):
            eng.tensor_tensor(out=PS3[:, :, a:b], in0=P4T3[:, 0:2, a:b],
                              in1=P4T3[:, 2:4, a:b], op=OP.add)
            eng.tensor_tensor(out=PAREA[:, a:b], in0=PS3[:, 0, a:b],
                              in1=PS3[:, 1, a:b], op=OP.mult)
            nc.vector.tensor_tensor(out=MIN43[:, :, a:b], in0=P4T3[:, :, a:b],
                                    in1=TLRB3[:, :, a:b], op=OP.min)
            eng.tensor_tensor(out=WIHI3[:, :, a:b], in0=MIN43[:, 0:2, a:b],
                              in1=MIN43[:, 2:4, a:b], op=OP.add)
            eng.tensor_tensor(out=AI[:, a:b], in0=WIHI3[:, 0, a:b],
                              in1=WIHI3[:, 1, a:b], op=OP.mult)
            eng.tensor_tensor(out=AU[:, a:b], in0=TAREA[:, a:b],
                              in1=PAREA[:, a:b], op=OP.add)
            eng.tensor_tensor(out=AU[:, a:b], in0=AU[:, a:b],
                              in1=AI[:, a:b], op=OP.subtract)
            nc.scalar.activation(LNAI[:, a:b], AI[:, a:b], AF.Ln, bias=CONSTEPS)
            nc.scalar.activation(LNAU[:, a:b], AU[:, a:b], AF.Ln, bias=CONSTEPS)
            eng.tensor_tensor(out=LNR[:, a:b], in0=LNAI[:, a:b],
                              in1=LNAU[:, a:b], op=OP.subtract)

        def tot_region(a, b):
            nc.vector.tensor_tensor(out=FCM[:, a:b], in0=F0W[:, a:b],
                                    in1=CONTR[:, a:b], op=OP.add)
            nc.vector.tensor_tensor(out=FCM[:, a:b], in0=FCM[:, a:b],
                                    in1=M74[:, a:b], op=OP.mult)
            nc.vector.tensor_tensor(out=LNM[:, a:b], in0=LNR[:, a:b],
                                    in1=M74[:, a:b], op=OP.mult)
            nc.vector.scalar_tensor_tensor(TOT[:, a:b], FCM[:, a:b], 0.75,
                                           LNM[:, a:b], OP.mult, OP.add)

        def sl_region(l):
            a, b = _lvl_cols[l], _lvl_cols[l + 1]
            nc.vector.tensor_reduce(SL5[:, l:l + 1], TOT[:, a:b],
                                    axis=AX.X, op=OP.add)

        L0A, L0B = _lvl_cols[0], _lvl_cols[1]
        L4B = _lvl_cols[5]
        # level-0 f0 per row-block (pipelines with its 5 gathers); psel-L1
        # one-hot issued early so the psel->contr->tot chain isn't the tail
        f0_region(0 * W[0], 1 * W[0])
        f0_region(1 * W[0], 2 * W[0])
        psel_region(0)
        contr_region(L0A, L0B)
        p4t_region(0, nc.vector)
        iou_region(L0A, L0B, nc.vector)
        for b in range(2, NH[0]):
            f0_region(b * W[0], (b + 1) * W[0])
        tot_region(L0A, L0B)
        sl_region(0)
        psel_region(1)
        f0_region(_lvl_cols[1], _lvl_cols[2])
        for l in range(2, NLEV):
            psel_region(l)
            f0_region(_lvl_cols[l], _lvl_cols[l + 1], eng_mult=nc.gpsimd)
        contr_region(L0B, L4B)
        for l in range(1, NLEV):
            p4t_region(l, nc.gpsimd)
        iou_region(L0B, L4B, nc.vector)
        tot_region(L0B, L4B)
        for l in range(1, NLEV):
            sl_region(l)

        # ---- combine halves, finalize loss, argmin --------------------------
        LVA = pc.tile([128, NLEV], f32)
        nc.vector.tensor_tensor(out=LVA[:], in0=SL5[:], in1=RECDN[:], op=OP.mult)
        SLH = pc.tile([64, NLEV], f32)
        nc.vector.stream_shuffle(SLH[:], LVA[64:128, :], list(range(32)))
        LOSSH0 = pc.tile([64, NLEV], f32)
        nc.vector.scalar_tensor_tensor(LOSSH0[:], EMX[0:64, :], BIG,
                                       LVA[0:64, :], OP.mult, OP.subtract)
        LOSS = pc.tile([64, NLEV], f32)
        nc.vector.tensor_tensor(out=LOSS[:], in0=LOSSH0[:], in1=SLH[:],
                                op=OP.subtract)
        MBEST = pc.tile([64, 1], f32)
        nc.vector.tensor_reduce(MBEST[:], LOSS[:], axis=AX.X, op=OP.min)
        EQ5 = pc.tile([64, NLEV], f32)
        nc.vector.tensor_tensor(out=EQ5[:], in0=LOSS[:],
                                in1=MBEST[:, 0:1].to_broadcast([64, NLEV]),
                                op=OP.is_equal)
        PEN5 = pc.tile([64, NLEV], f32)
        nc.vector.scalar_tensor_tensor(PEN5[:], EQ5[:], -BIG2, CLB5[0:64, :],
                                       OP.mult, OP.add)
        IDX = pc.tile([64, 1], f32)
        nc.vector.tensor_reduce(IDX[:], PEN5[:], axis=AX.X, op=OP.min)
        nc.vector.copy_predicated(IDX[:], NV[0:64, :], CONSTM1[0:64, :])
        IDXI = pc.tile([64, 1], i32)
        nc.vector.tensor_copy(IDXI[:], IDX[:])
        nc.sync.dma_start(out_lvl.ap()[:, None], IDXI[:])

    nc.compile()
    return nc


_NC_CACHE = None


def _get_nc():
    global _NC_CACHE
    if _NC_CACHE is None:
        _NC_CACHE = build_nc(num_devices=8)
    return _NC_CACHE


def _host_idx(gt):
    """Per-image gather descriptors (pure [64,5] geometry metadata):
    row-block start offsets into the combined array + the psel span start."""
    boxes = gt[:, 0:4].astype(np.float64)
    label = np.clip(gt[:, 4].astype(np.int64), 0, C - 1)
    cx = (boxes[:, 0] + boxes[:, 2]) * 0.5
    cy = (boxes[:, 1] + boxes[:, 3]) * 0.5
    hw = (boxes[:, 2] - boxes[:, 0]) * 0.5 * 0.2
    hh = (boxes[:, 3] - boxes[:, 1]) * 0.5 * 0.2
    # float32 arithmetic to match the device V-chain exactly
    qx1 = (np.float32(0.6) * boxes[:, 0].astype(np.float32)
           + np.float32(0.4) * boxes[:, 2].astype(np.float32))
    qy1 = (np.float32(0.6) * boxes[:, 1].astype(np.float32)
           + np.float32(0.4) * boxes[:, 3].astype(np.float32))
    out = np.zeros((128, 14), np.int32)
    for s, (l, b, k) in enumerate(_J0):
        fh, fw = FS[l]
        w = W[l]
        v1x = (qx1 * np.float32(1.0 / STRIDES[l])).astype(np.float32)
        v1y = (qy1 * np.float32(1.0 / STRIDES[l])).astype(np.float32)
        xs = np.clip(np.floor(v1x), 0, fw - 1)
        ys = np.clip(np.floor(v1y), 0, fh - 1)
        xs = np.minimum(xs, fw - w)
        ys_lo = np.minimum(ys + b, fh - 1)
        ys_hi = np.minimum(ys + NH[l] + b, fh - 1)
        out[0:64, s] = ((ys_lo * fw + xs + ROWOFS[l]) * CC).astype(np.int32)
        out[64:128, s] = ((ys_hi * fw + xs + ROWOFS[l]) * CC).astype(np.int32)
    fh, fw = FS[0]
    v1x = (qx1 * np.float32(1.0 / STRIDES[0])).astype(np.float32)
    v1y = (qy1 * np.float32(1.0 / STRIDES[0])).astype(np.float32)
    xs = np.minimum(np.clip(np.floor(v1x), 0, fw - 1), fw - W[0])
    ys = np.minimum(np.clip(np.floor(v1y), 0, fh - 1), fh - W[0])
    st = label * NLOC + (ys * fw + xs).astype(np.int64)
    out[0:64, 12] = st.astype(np.int32)
    out[64:128, 12] = (st + NH[0] * fw).astype(np.int32)
    fh, fw = FS[1]
    v1x = (qx1 * np.float32(1.0 / STRIDES[1])).astype(np.float32)
    v1y = (qy1 * np.float32(1.0 / STRIDES[1])).astype(np.float32)
    xs = np.minimum(np.clip(np.floor(v1x), 0, fw - 1), fw - W[1])
    ys = np.minimum(np.clip(np.floor(v1y), 0, fh - 1), fh - W[1])
    st = label * NLOC + ROWOFS[1] + (ys * fw + xs).astype(np.int64)
    out[0:64, 13] = st.astype(np.int32)
    out[64:128, 13] = (st + NH[1] * fw).astype(np.int32)
    return out


def kernel(cls_pred, regr_pred, feature_shapes, gt_boxes):
    from concourse.bass_utils import run_bass_kernel_spmd

    B = cls_pred.shape[0]
    assert B == 8 and cls_pred.shape[1] == NLOC and cls_pred.shape[2] == C
    nc = _get_nc()
    cls_pred = np.asarray(cls_pred, dtype=np.float32)
    regr_pred = np.asarray(regr_pred, dtype=np.float32)
    gt_boxes = np.asarray(gt_boxes, dtype=np.float32)
    in_maps = [
        {
            "comb_b": np.ascontiguousarray(
                np.concatenate([cls_pred[b], regr_pred[b]], axis=-1)),
            "clsT_b": np.ascontiguousarray(cls_pred[b].T),
            "gt_b": np.ascontiguousarray(gt_boxes[b]),
            "idx_b": _host_idx(gt_boxes[b]),
        }
        for b in range(B)
    ]
    res = run_bass_kernel_spmd(nc, in_maps, list(range(B)))
    out = np.stack([np.asarray(res.results[b]["out_lvl"]).reshape(G)
                    for b in range(B)])
    return out.reshape(-1).astype(np.int32)
